# revision 19
# baseline (speedup 1.0000x reference)
"""MoE experts kernel for Trainium2 (8 NeuronCores, expert-parallel).

Reference computation (per token t, top-k expert e with gate p):
    y[t] = sum_k p[t,k] * down_e @ (silu(x[t] @ gate_e) * (x[t] @ up_e))
with per-expert capacity CAP=1024 (tokens beyond capacity dropped).

Strategy:
  - Host: sort token assignments by expert (stable, matching jnp.argsort),
    build per-expert dense token buffers transposed to [H, NPAD] so the
    device kernel needs no transposes anywhere.
  - Device (SPMD over 8 cores, 8 experts/core): grouped GEMMs in float32r
    (full-rate PE fp32 mode, ~1e-4 rel err):
       G^T = gate^T-slices @ X^T   (accumulate over H chunks)
       U^T = up^T-slices   @ X^T
       Hm^T = silu(G^T) * U^T
       O^T  = down^T-slices @ Hm^T (accumulate over I chunks)
  - Host: gather rows back, apply routing weights, sum over top-k.
"""

import os
import sys

sys.path.insert(0, "/opt/trn_rl_repo")

import numpy as np

E, H, I, T, K = 64, 2048, 768, 4096, 8
CAP = 1024
NCORES = 8
EPC = E // NCORES  # experts per core
NH = H // 128  # 16 contraction chunks for gate/up
NI = I // 128  # 6 contraction chunks for down

_prog_cache = {}
LAST_EXEC_NS = None
LAST_RESULTS = None


def _groups(npad):
    ng = -(-npad // 512)
    w = -(-npad // ng)
    out = []
    s = 0
    while s < npad:
        e = min(s + w, npad)
        out.append((s, e))
        s = e
    return out


MM_DT = "float16"  # matmul operand dtype: float16 | float32r


def _build_program(slotw):
    import concourse.bacc as bacc
    import concourse.mybir as mybir
    from concourse.tile import TileContext

    f32 = mybir.dt.float32
    mdt = getattr(mybir.dt, MM_DT)
    SILU = mybir.ActivationFunctionType.Silu

    nc = bacc.Bacc(None, target_bir_lowering=False)
    xTs = [
        nc.declare_dram_parameter(f"xT{j}", [NH, 128, w], mdt, isOutput=False)
        for j, w in enumerate(slotw)
    ]
    gw = nc.declare_dram_parameter("gw", [EPC, NI, 128, NH, 128], mdt, isOutput=False)
    uw = nc.declare_dram_parameter("uw", [EPC, NI, 128, NH, 128], mdt, isOutput=False)
    dw = nc.declare_dram_parameter("dw", [EPC, NH, 128, NI, 128], mdt, isOutput=False)
    yTs = [
        nc.declare_dram_parameter(f"yT{j}", [NH, 128, w], f32, isOutput=True)
        for j, w in enumerate(slotw)
    ]

    xt_bufs = 2

    with TileContext(nc) as tc:
        with (
            tc.sbuf_pool(name="xp", bufs=xt_bufs) as xp,
            tc.sbuf_pool(name="wp", bufs=3) as wp,
            tc.sbuf_pool(name="hp", bufs=2) as hp,
            tc.sbuf_pool(name="op", bufs=3) as op,
            tc.sbuf_pool(name="tp", bufs=3) as tp,
            tc.psum_pool(name="pp", bufs=2) as pp,
        ):
            for e in range(EPC):
                npad = slotw[e]
                groups = _groups(npad)
                g_w0 = wp.tile([128, NH, 128], mdt, name="g_w", tag="g_w")
                u_w0 = wp.tile([128, NH, 128], mdt, name="u_w", tag="u_w")
                xts = []
                xr = xTs[e][:, :, :].rearrange("h p n -> p h n")
                if e == 0:
                    # fine-grained first loads: the first LDW/MM should wait on
                    # ~0.13MB, not the whole 2MB burst
                    xt_t0 = xp.tile([128, 4, npad], mdt, name="xt0", tag="xt0")
                    nc.sync.dma_start(out=g_w0[:, 0:2, :], in_=gw[e, 0, :, 0:2, :])
                    nc.sync.dma_start(out=xt_t0[:, 0:1, :], in_=xr[:, 0:1, :])
                    nc.sync.dma_start(out=g_w0[:, 2:NH, :], in_=gw[e, 0, :, 2:NH, :])
                    nc.sync.dma_start(out=xt_t0[:, 1:4, :], in_=xr[:, 1:4, :])
                    nc.sync.dma_start(out=u_w0, in_=uw[e, 0, :, :, :])
                    xts.extend(xt_t0[:, jj, :] for jj in range(4))
                    for j in range(1, 4):
                        xt_t = xp.tile([128, 4, npad], mdt, name=f"xt{j}", tag=f"xt{j}")
                        nc.sync.dma_start(out=xt_t, in_=xr[:, 4 * j : 4 * (j + 1), :])
                        xts.extend(xt_t[:, jj, :] for jj in range(4))
                else:
                    nc.sync.dma_start(out=g_w0, in_=gw[e, 0, :, :, :])
                    nc.sync.dma_start(out=u_w0, in_=uw[e, 0, :, :, :])
                    for j in range(4):
                        xt_t = xp.tile([128, 4, npad], mdt, name=f"xt{j}", tag=f"xt{j}")
                        nc.sync.dma_start(out=xt_t, in_=xr[:, 4 * j : 4 * (j + 1), :])
                        xts.extend(xt_t[:, jj, :] for jj in range(4))
                hms = [
                    hp.tile([128, npad], mdt, name=f"hm{i}", tag=f"hm{i}")
                    for i in range(NI)
                ]
                for i in range(NI):
                    if i == 0:
                        g_w, u_w = g_w0, u_w0
                    else:
                        g_w = wp.tile([128, NH, 128], mdt, name="g_w", tag="g_w")
                        u_w = wp.tile([128, NH, 128], mdt, name="u_w", tag="u_w")
                        nc.sync.dma_start(out=g_w, in_=gw[e, i, :, :, :])
                        nc.sync.dma_start(out=u_w, in_=uw[e, i, :, :, :])
                    for g0, g1 in groups:
                        wdt = g1 - g0
                        psg = pp.tile([128, wdt], f32, name="psg", tag="psg", bufs=3)
                        psu = pp.tile([128, wdt], f32, name="psu", tag="psu", bufs=2)
                        for h in range(NH):
                            nc.tensor.matmul(
                                psg,
                                g_w[:, h, :],
                                xts[h][:, g0:g1],
                                start=(h == 0),
                                stop=(h == NH - 1),
                            )
                        for h in range(NH):
                            nc.tensor.matmul(
                                psu,
                                u_w[:, h, :],
                                xts[h][:, g0:g1],
                                start=(h == 0),
                                stop=(h == NH - 1),
                            )
                        sil = tp.tile([128, wdt], f32, name="sil", tag="sil")
                        nc.scalar.activation(sil, psg, SILU)
                        nc.vector.tensor_mul(hms[i][:, g0:g1], sil, psu)
                d_w = wp.tile([128, NH, NI, 128], mdt, name="d_w", tag="d_w", bufs=2)
                nc.sync.dma_start(out=d_w, in_=dw[e].rearrange("h p i m -> p h i m"))
                for h in range(NH):
                    ot = op.tile([128, npad], f32, name="ot", tag="ot", bufs=6)
                    for gi, (g0, g1) in enumerate(groups):
                        wdt = g1 - g0
                        pso = pp.tile([128, wdt], f32, name="pso", tag="pso")
                        for i in range(NI):
                            nc.tensor.matmul(
                                pso,
                                d_w[:, h, i, :],
                                hms[i][:, g0:g1],
                                start=(i == 0),
                                stop=(i == NI - 1),
                            )
                        nc.vector.tensor_copy(ot[:, g0:g1], pso)
                    nc.sync.dma_start(out=yTs[e][h, :, :], in_=ot)
    nc.compile()
    return nc


def _install_neff_cache():
    """Cache walrus NEFF compiles on disk keyed by BIR hash (compile of the
    ~11k-instruction program takes minutes; the BIR is deterministic)."""
    import hashlib
    import shutil

    import concourse.bass2jax as bass2jax
    from concourse.bass_utils import compile_bir_kernel as _orig

    if getattr(bass2jax.compile_bir_kernel, "_moe_cached", False):
        return
    cache_dir = os.environ.get("BASS_NEFF_CACHE", "/tmp/bass_neff_cache")
    os.makedirs(cache_dir, exist_ok=True)

    def cached(bir_json, tmpdir, neff_name="file.neff"):
        key = hashlib.sha256(bir_json).hexdigest()[:24]
        cpath = os.path.join(cache_dir, key + ".neff")
        dst = os.path.join(tmpdir, neff_name)
        if os.path.exists(cpath):
            shutil.copy(cpath, dst)
            return dst
        out = _orig(bir_json, tmpdir, neff_name)
        try:
            shutil.copy(out, cpath)
        except OSError:
            pass
        return out

    cached._moe_cached = True
    bass2jax.compile_bir_kernel = cached


def _install_ntff_hook_shim():
    """Provide antenv.axon_hooks (absent in this container) so that
    run_bass_kernel_spmd(trace=True) can capture NTFF profiles via the
    axon .so — mirrors trn_agent_boot.trn_boot's ctypes hook."""
    import types
    import ctypes
    import contextlib

    if "antenv.axon_hooks" in sys.modules:
        return
    so_path = "/opt/axon/libaxon_pjrt.so"
    lib = ctypes.CDLL(so_path)
    if not hasattr(lib, "axon_start_nrt_profile"):
        return
    lib.axon_start_nrt_profile.argtypes = [
        ctypes.POINTER(ctypes.c_int64),
        ctypes.c_size_t,
    ]
    lib.axon_start_nrt_profile.restype = ctypes.c_int64
    lib.axon_stop_nrt_profile.argtypes = [ctypes.c_char_p]
    lib.axon_stop_nrt_profile.restype = ctypes.c_int64

    @contextlib.contextmanager
    def _hook(output_dir, device_ids):
        import jax

        jax.devices()
        if device_ids:
            ids = (ctypes.c_int64 * len(device_ids))(*device_ids)
            rc = lib.axon_start_nrt_profile(ids, len(device_ids))
        else:
            rc = lib.axon_start_nrt_profile(None, 0)
        if rc != 0:
            raise RuntimeError(f"axon_start_nrt_profile rc={rc}")
        try:
            yield
        finally:
            n = lib.axon_stop_nrt_profile(str(output_dir).encode())
            print(f"profile: {n} file(s) written to {output_dir}", file=sys.stderr)

    mod = types.ModuleType("antenv.axon_hooks")
    mod.get_axon_ntff_profile_hook = lambda: _hook
    mod.set_axon_ntff_profile_hook = lambda h: None
    sys.modules["antenv.axon_hooks"] = mod


def kernel(
    hidden_states,
    routing_weights,
    selected_experts,
    gate_proj,
    up_proj,
    down_proj,
):
    global LAST_EXEC_NS, LAST_RESULTS
    from concourse.bass_utils import run_bass_kernel_spmd

    _install_neff_cache()

    x = np.ascontiguousarray(np.asarray(hidden_states, dtype=np.float32))
    rw = np.asarray(routing_weights, dtype=np.float32)
    sel = np.asarray(selected_experts).astype(np.int64)
    gate = np.asarray(gate_proj, dtype=np.float32)
    up = np.asarray(up_proj, dtype=np.float32)
    down = np.asarray(down_proj, dtype=np.float32)

    # ---- host dispatch (mirrors reference's stable sort-by-expert) ----
    flat_e = sel.reshape(-1)
    order = np.argsort(flat_e, kind="stable")
    sorted_e = flat_e[order]
    counts = np.bincount(flat_e, minlength=E)
    offsets = np.concatenate([[0], np.cumsum(counts)[:-1]])
    pos = np.arange(flat_e.shape[0], dtype=np.int64) - offsets[sorted_e]

    # ---- slot assignment: per core, experts sorted by load (desc); slot j's
    # compile-time width = max over cores, rounded to 64, capped at CAP ----
    perm = np.zeros((NCORES, EPC), dtype=np.int64)  # perm[c, j] = expert id
    for c in range(NCORES):
        ids = np.arange(c * EPC, (c + 1) * EPC)
        perm[c] = ids[np.argsort(-counts[ids], kind="stable")]
    wmin = 256 if MM_DT == "float32r" else 64
    slotw = tuple(
        int(min(CAP, max(wmin, -(-int(counts[perm[:, j]].max()) // 32) * 32)))
        for j in range(EPC)
    )
    w_of_expert = np.zeros(E, dtype=np.int64)
    for c in range(NCORES):
        for j in range(EPC):
            w_of_expert[perm[c, j]] = slotw[j]

    keep = pos < w_of_expert[sorted_e]  # width >= min(count, CAP); drops only > CAP

    tok = order // K
    ke = sorted_e[keep]
    kp = pos[keep]

    # Dense per-expert buffers, transposed: xbufT[e] = X_e^T  [H, w_e]
    maxw = max(slotw)
    xbuf = np.zeros((E, maxw, H), dtype=np.float32)
    xbuf[ke, kp] = x[tok[keep]]

    # ---- weight/token layouts (contiguous per-DMA blocks) ----
    # gate/up slice for (e, i): [128p, NH, 128c] where [p, h, c] = W[h*128+p, i*128+c]
    gate_r = gate.reshape(E, NH, 128, NI, 128).transpose(0, 3, 2, 1, 4)
    up_r = up.reshape(E, NH, 128, NI, 128).transpose(0, 3, 2, 1, 4)
    # down slice for (e, h): [128p, NI, 128m] where [p, i, m] = W[i*128+p, h*128+m]
    down_r = down.reshape(E, NI, 128, NH, 128).transpose(0, 3, 2, 1, 4)

    nc = _prog_cache.get(slotw)
    if nc is None:
        nc = _build_program(slotw)
        _prog_cache[slotw] = nc

    mm_np = np.float16 if MM_DT == "float16" else np.float32
    in_maps = []
    for c in range(NCORES):
        m = {
            "gw": np.ascontiguousarray(gate_r[perm[c]], dtype=mm_np),
            "uw": np.ascontiguousarray(up_r[perm[c]], dtype=mm_np),
            "dw": np.ascontiguousarray(down_r[perm[c]], dtype=mm_np),
        }
        for j in range(EPC):
            e = perm[c, j]
            w = slotw[j]
            # [H, w] -> [NH, 128, w]
            m[f"xT{j}"] = np.ascontiguousarray(
                xbuf[e, :w].T.reshape(NH, 128, w), dtype=mm_np
            )
        in_maps.append(m)

    trace = bool(os.environ.get("BASS_MOE_TRACE"))
    kwargs = {}
    if trace:
        _install_ntff_hook_shim()
        tcores = os.environ.get("BASS_MOE_TRACE_CORES", "0")
        kwargs = dict(trace=True, trace_cores=[int(c) for c in tcores.split(",")])
    res = run_bass_kernel_spmd(nc, in_maps, core_ids=list(range(NCORES)), **kwargs)
    LAST_EXEC_NS = res.exec_time_ns
    LAST_RESULTS = res

    # ---- host combine ----
    # per expert e at (core c, slot j): yT{j} is [NH, 128, w] = O_e^T
    o_all = np.zeros((E, maxw, H), dtype=np.float32)
    for c in range(NCORES):
        for j in range(EPC):
            e = perm[c, j]
            w = slotw[j]
            o_all[e, :w] = res.results[c][f"yT{j}"].reshape(H, w).T

    gathered = np.zeros((flat_e.shape[0], H), dtype=np.float32)
    gathered[order[keep]] = o_all[ke, kp]
    y = (gathered.reshape(T, K, H) * rw[:, :, None]).sum(axis=1, dtype=np.float32)
    return y.astype(np.float32)


# revision 20
# speedup vs baseline: 1.0092x; 1.0092x over previous
"""MoE experts kernel for Trainium2 (8 NeuronCores, expert-parallel).

Reference computation (per token t, top-k expert e with gate p):
    y[t] = sum_k p[t,k] * down_e @ (silu(x[t] @ gate_e) * (x[t] @ up_e))
with per-expert capacity CAP=1024 (tokens beyond capacity dropped).

Strategy:
  - Host: sort token assignments by expert (stable, matching jnp.argsort),
    build per-expert dense token buffers transposed to [H, NPAD] so the
    device kernel needs no transposes anywhere.
  - Device (SPMD over 8 cores, 8 experts/core): grouped GEMMs in float32r
    (full-rate PE fp32 mode, ~1e-4 rel err):
       G^T = gate^T-slices @ X^T   (accumulate over H chunks)
       U^T = up^T-slices   @ X^T
       Hm^T = silu(G^T) * U^T
       O^T  = down^T-slices @ Hm^T (accumulate over I chunks)
  - Host: gather rows back, apply routing weights, sum over top-k.
"""

import os
import sys

sys.path.insert(0, "/opt/trn_rl_repo")

import numpy as np

E, H, I, T, K = 64, 2048, 768, 4096, 8
CAP = 1024
NCORES = 8
EPC = E // NCORES  # experts per core
NH = H // 128  # 16 contraction chunks for gate/up
NI = I // 128  # 6 contraction chunks for down

_prog_cache = {}
LAST_EXEC_NS = None
LAST_RESULTS = None


def _groups(npad):
    ng = -(-npad // 512)
    w = -(-npad // ng)
    out = []
    s = 0
    while s < npad:
        e = min(s + w, npad)
        out.append((s, e))
        s = e
    return out


MM_DT = "float16"  # matmul operand dtype: float16 | float32r


def _build_program(slotw):
    import concourse.bacc as bacc
    import concourse.mybir as mybir
    from concourse.tile import TileContext

    f32 = mybir.dt.float32
    mdt = getattr(mybir.dt, MM_DT)
    SILU = mybir.ActivationFunctionType.Silu

    nc = bacc.Bacc(None, target_bir_lowering=False)
    xTs = [
        nc.declare_dram_parameter(f"xT{j}", [NH, 128, w], mdt, isOutput=False)
        for j, w in enumerate(slotw)
    ]
    gw = nc.declare_dram_parameter("gw", [EPC, NI, 128, NH, 128], mdt, isOutput=False)
    uw = nc.declare_dram_parameter("uw", [EPC, NI, 128, NH, 128], mdt, isOutput=False)
    dw = nc.declare_dram_parameter("dw", [EPC, NH, 128, NI, 128], mdt, isOutput=False)
    yTs = [
        nc.declare_dram_parameter(f"yT{j}", [NH, 128, w], f32, isOutput=True)
        for j, w in enumerate(slotw)
    ]

    xt_bufs = 2

    with TileContext(nc) as tc:
        with (
            tc.sbuf_pool(name="xp", bufs=xt_bufs) as xp,
            tc.sbuf_pool(name="wp", bufs=3) as wp,
            tc.sbuf_pool(name="hp", bufs=2) as hp,
            tc.sbuf_pool(name="op", bufs=3) as op,
            tc.sbuf_pool(name="tp", bufs=3) as tp,
            tc.psum_pool(name="pp", bufs=2) as pp,
        ):
            for e in range(EPC):
                npad = slotw[e]
                groups = _groups(npad)
                g_w0 = wp.tile([128, NH, 128], mdt, name="g_w", tag="g_w")
                u_w0 = wp.tile([128, NH, 128], mdt, name="u_w", tag="u_w")
                xts = []
                xr = xTs[e][:, :, :].rearrange("h p n -> p h n")
                nc.sync.dma_start(out=g_w0, in_=gw[e, 0, :, :, :])
                nc.sync.dma_start(out=u_w0, in_=uw[e, 0, :, :, :])
                for j in range(4):
                    xt_t = xp.tile([128, 4, npad], mdt, name=f"xt{j}", tag=f"xt{j}")
                    nc.sync.dma_start(out=xt_t, in_=xr[:, 4 * j : 4 * (j + 1), :])
                    xts.extend(xt_t[:, jj, :] for jj in range(4))
                hms = [
                    hp.tile([128, npad], mdt, name=f"hm{i}", tag=f"hm{i}")
                    for i in range(NI)
                ]
                for i in range(NI):
                    if i == 0:
                        g_w, u_w = g_w0, u_w0
                    else:
                        g_w = wp.tile([128, NH, 128], mdt, name="g_w", tag="g_w")
                        u_w = wp.tile([128, NH, 128], mdt, name="u_w", tag="u_w")
                        nc.sync.dma_start(out=g_w, in_=gw[e, i, :, :, :])
                        nc.sync.dma_start(out=u_w, in_=uw[e, i, :, :, :])
                    for g0, g1 in groups:
                        wdt = g1 - g0
                        psg = pp.tile([128, wdt], f32, name="psg", tag="psg", bufs=3)
                        psu = pp.tile([128, wdt], f32, name="psu", tag="psu", bufs=2)
                        for h in range(NH):
                            nc.tensor.matmul(
                                psg,
                                g_w[:, h, :],
                                xts[h][:, g0:g1],
                                start=(h == 0),
                                stop=(h == NH - 1),
                            )
                        for h in range(NH):
                            nc.tensor.matmul(
                                psu,
                                u_w[:, h, :],
                                xts[h][:, g0:g1],
                                start=(h == 0),
                                stop=(h == NH - 1),
                            )
                        sil = tp.tile([128, wdt], f32, name="sil", tag="sil")
                        nc.scalar.activation(sil, psg, SILU)
                        nc.vector.tensor_mul(hms[i][:, g0:g1], sil, psu)
                d_w = wp.tile([128, NH, NI, 128], mdt, name="d_w", tag="d_w", bufs=2)
                nc.sync.dma_start(out=d_w, in_=dw[e].rearrange("h p i m -> p h i m"))
                for h in range(NH):
                    ot = op.tile([128, npad], f32, name="ot", tag="ot", bufs=6)
                    for gi, (g0, g1) in enumerate(groups):
                        wdt = g1 - g0
                        pso = pp.tile([128, wdt], f32, name="pso", tag="pso")
                        for i in range(NI):
                            nc.tensor.matmul(
                                pso,
                                d_w[:, h, i, :],
                                hms[i][:, g0:g1],
                                start=(i == 0),
                                stop=(i == NI - 1),
                            )
                        nc.vector.tensor_copy(ot[:, g0:g1], pso)
                    nc.sync.dma_start(out=yTs[e][h, :, :], in_=ot)
    nc.compile()
    return nc


def _install_neff_cache():
    """Cache walrus NEFF compiles on disk keyed by BIR hash (compile of the
    ~11k-instruction program takes minutes; the BIR is deterministic)."""
    import hashlib
    import shutil

    import concourse.bass2jax as bass2jax
    from concourse.bass_utils import compile_bir_kernel as _orig

    if getattr(bass2jax.compile_bir_kernel, "_moe_cached", False):
        return
    cache_dir = os.environ.get("BASS_NEFF_CACHE", "/tmp/bass_neff_cache")
    os.makedirs(cache_dir, exist_ok=True)

    def cached(bir_json, tmpdir, neff_name="file.neff"):
        key = hashlib.sha256(bir_json).hexdigest()[:24]
        cpath = os.path.join(cache_dir, key + ".neff")
        dst = os.path.join(tmpdir, neff_name)
        if os.path.exists(cpath):
            shutil.copy(cpath, dst)
            return dst
        out = _orig(bir_json, tmpdir, neff_name)
        try:
            shutil.copy(out, cpath)
        except OSError:
            pass
        return out

    cached._moe_cached = True
    bass2jax.compile_bir_kernel = cached


def _install_ntff_hook_shim():
    """Provide antenv.axon_hooks (absent in this container) so that
    run_bass_kernel_spmd(trace=True) can capture NTFF profiles via the
    axon .so — mirrors trn_agent_boot.trn_boot's ctypes hook."""
    import types
    import ctypes
    import contextlib

    if "antenv.axon_hooks" in sys.modules:
        return
    so_path = "/opt/axon/libaxon_pjrt.so"
    lib = ctypes.CDLL(so_path)
    if not hasattr(lib, "axon_start_nrt_profile"):
        return
    lib.axon_start_nrt_profile.argtypes = [
        ctypes.POINTER(ctypes.c_int64),
        ctypes.c_size_t,
    ]
    lib.axon_start_nrt_profile.restype = ctypes.c_int64
    lib.axon_stop_nrt_profile.argtypes = [ctypes.c_char_p]
    lib.axon_stop_nrt_profile.restype = ctypes.c_int64

    @contextlib.contextmanager
    def _hook(output_dir, device_ids):
        import jax

        jax.devices()
        if device_ids:
            ids = (ctypes.c_int64 * len(device_ids))(*device_ids)
            rc = lib.axon_start_nrt_profile(ids, len(device_ids))
        else:
            rc = lib.axon_start_nrt_profile(None, 0)
        if rc != 0:
            raise RuntimeError(f"axon_start_nrt_profile rc={rc}")
        try:
            yield
        finally:
            n = lib.axon_stop_nrt_profile(str(output_dir).encode())
            print(f"profile: {n} file(s) written to {output_dir}", file=sys.stderr)

    mod = types.ModuleType("antenv.axon_hooks")
    mod.get_axon_ntff_profile_hook = lambda: _hook
    mod.set_axon_ntff_profile_hook = lambda h: None
    sys.modules["antenv.axon_hooks"] = mod


def kernel(
    hidden_states,
    routing_weights,
    selected_experts,
    gate_proj,
    up_proj,
    down_proj,
):
    global LAST_EXEC_NS, LAST_RESULTS
    from concourse.bass_utils import run_bass_kernel_spmd

    _install_neff_cache()

    x = np.ascontiguousarray(np.asarray(hidden_states, dtype=np.float32))
    rw = np.asarray(routing_weights, dtype=np.float32)
    sel = np.asarray(selected_experts).astype(np.int64)
    gate = np.asarray(gate_proj, dtype=np.float32)
    up = np.asarray(up_proj, dtype=np.float32)
    down = np.asarray(down_proj, dtype=np.float32)

    # ---- host dispatch (mirrors reference's stable sort-by-expert) ----
    flat_e = sel.reshape(-1)
    order = np.argsort(flat_e, kind="stable")
    sorted_e = flat_e[order]
    counts = np.bincount(flat_e, minlength=E)
    offsets = np.concatenate([[0], np.cumsum(counts)[:-1]])
    pos = np.arange(flat_e.shape[0], dtype=np.int64) - offsets[sorted_e]

    # ---- slot assignment: per core, experts sorted by load (desc); slot j's
    # compile-time width = max over cores, rounded to 64, capped at CAP ----
    perm = np.zeros((NCORES, EPC), dtype=np.int64)  # perm[c, j] = expert id
    for c in range(NCORES):
        ids = np.arange(c * EPC, (c + 1) * EPC)
        perm[c] = ids[np.argsort(-counts[ids], kind="stable")]
    wmin = 256 if MM_DT == "float32r" else 64
    slotw = tuple(
        int(min(CAP, max(wmin, -(-int(counts[perm[:, j]].max()) // 32) * 32)))
        for j in range(EPC)
    )
    w_of_expert = np.zeros(E, dtype=np.int64)
    for c in range(NCORES):
        for j in range(EPC):
            w_of_expert[perm[c, j]] = slotw[j]

    keep = pos < w_of_expert[sorted_e]  # width >= min(count, CAP); drops only > CAP

    tok = order // K
    ke = sorted_e[keep]
    kp = pos[keep]

    # Dense per-expert buffers, transposed: xbufT[e] = X_e^T  [H, w_e]
    maxw = max(slotw)
    xbuf = np.zeros((E, maxw, H), dtype=np.float32)
    xbuf[ke, kp] = x[tok[keep]]

    # ---- weight/token layouts (contiguous per-DMA blocks) ----
    # gate/up slice for (e, i): [128p, NH, 128c] where [p, h, c] = W[h*128+p, i*128+c]
    gate_r = gate.reshape(E, NH, 128, NI, 128).transpose(0, 3, 2, 1, 4)
    up_r = up.reshape(E, NH, 128, NI, 128).transpose(0, 3, 2, 1, 4)
    # down slice for (e, h): [128p, NI, 128m] where [p, i, m] = W[i*128+p, h*128+m]
    down_r = down.reshape(E, NI, 128, NH, 128).transpose(0, 3, 2, 1, 4)

    nc = _prog_cache.get(slotw)
    if nc is None:
        nc = _build_program(slotw)
        _prog_cache[slotw] = nc

    mm_np = np.float16 if MM_DT == "float16" else np.float32
    in_maps = []
    for c in range(NCORES):
        m = {
            "gw": np.ascontiguousarray(gate_r[perm[c]], dtype=mm_np),
            "uw": np.ascontiguousarray(up_r[perm[c]], dtype=mm_np),
            "dw": np.ascontiguousarray(down_r[perm[c]], dtype=mm_np),
        }
        for j in range(EPC):
            e = perm[c, j]
            w = slotw[j]
            # [H, w] -> [NH, 128, w]
            m[f"xT{j}"] = np.ascontiguousarray(
                xbuf[e, :w].T.reshape(NH, 128, w), dtype=mm_np
            )
        in_maps.append(m)

    trace = bool(os.environ.get("BASS_MOE_TRACE"))
    kwargs = {}
    if trace:
        _install_ntff_hook_shim()
        tcores = os.environ.get("BASS_MOE_TRACE_CORES", "0")
        kwargs = dict(trace=True, trace_cores=[int(c) for c in tcores.split(",")])
    res = run_bass_kernel_spmd(nc, in_maps, core_ids=list(range(NCORES)), **kwargs)
    LAST_EXEC_NS = res.exec_time_ns
    LAST_RESULTS = res

    # ---- host combine ----
    # per expert e at (core c, slot j): yT{j} is [NH, 128, w] = O_e^T
    o_all = np.zeros((E, maxw, H), dtype=np.float32)
    for c in range(NCORES):
        for j in range(EPC):
            e = perm[c, j]
            w = slotw[j]
            o_all[e, :w] = res.results[c][f"yT{j}"].reshape(H, w).T

    gathered = np.zeros((flat_e.shape[0], H), dtype=np.float32)
    gathered[order[keep]] = o_all[ke, kp]
    y = (gathered.reshape(T, K, H) * rw[:, :, None]).sum(axis=1, dtype=np.float32)
    return y.astype(np.float32)
